# revision 12
# baseline (speedup 1.0000x reference)
"""CRF negative-log-likelihood kernel for Trainium2, SPMD over 8 NeuronCores.

Strategy (v2)
-------------
Data-parallel over batch: core c handles sequences b in [c*8, (c+1)*8).

Per core (B=8 local sequences, T=512, K=50 tags, D=1024):

1. Emissions GEMM in bf16 from HOST-pre-transposed hidden (hidT packed
   [p, seq, dchunk, t] so DMA lines are 8KB-contiguous and no on-device
   transpose is needed).  Per sequence: 8 accumulating matmuls
   [128 x 50 x 512] -> PSUM emis [50, 512].
2. E' build (renorm-free scan): E_raw = exp(emis + b) (ScalarE, bf16);
   cs = s0 * colsum(E_raw) via ones-matmul; E' = E_raw * (1/cs)
   (reciprocal broadcast over partitions with a rank-1 matmul).  The
   per-column log corrections ln(cs) accumulate off the critical path:
   log_Z = ln(w . a) + sum_t ln(cs_t).  With s0 = mean(exp(transitions))
   the scaled recurrence drifts only O(sqrt(T) * 0.02) e-folds: no
   renormalization needed inside the scan at all.
3. Partition function with HALVED serial depth: split the matrix-product
   chain in the middle,
       log_Z = ln( w . a ),
       a = A_255 ... A_1 alpha_0          (forward chain,  255 steps)
       w = A_256^T ... A_511^T exp(end)   (backward chain, 256 steps)
   where A_t = diag(E'_t) M^T.  Forward step: PE matmul (M as lhsT) then
   DVE multiply by E'_t.  Backward step: DVE multiply by E'_t then PE
   matmul (M^T as lhsT).  The two chains are independent and ping-pong
   PE<->DVE concurrently; everything is bf16 single-pass on the PE.
4. Gold score: emission part on device via ONE scalar_tensor_tensor per
   sequence: out = (bcast(tags) == iota) * emis with accum_out giving
   the per-tag sums; a ones-matmul reduces over tags.  The transition +
   start/end part is a pure function of tag_ids, computed on host.
"""

import numpy as np

B_FULL = 64
B_LOC = 8
T = 512
K = 50
D = 1024
DC = 8  # d chunks of 128
N_CORES = 8
BT = B_LOC * T  # 4096
MID = 256  # fwd handles t=1..255, bwd t=511..256

_COMPILED = {}
LAST_RESULT = None


def _build():
    import concourse.bass as bass
    import concourse.tile as tile
    from concourse import bacc, mybir

    f32 = mybir.dt.float32
    bf16 = mybir.dt.bfloat16

    nc = bacc.Bacc(
        "TRN2",
        target_bir_lowering=False,
        debug=False,
        num_devices=N_CORES,
    )

    hidT = nc.dram_tensor("hidT", [128, B_LOC, DC, T], bf16, kind="ExternalInput")
    wq = nc.dram_tensor("wq", [128, DC, K], bf16, kind="ExternalInput")
    mf = nc.dram_tensor("mf", [K, K], bf16, kind="ExternalInput")
    mb = nc.dram_tensor("mb", [K, K], bf16, kind="ExternalInput")
    tagr = nc.dram_tensor("tagr", [1, BT], bf16, kind="ExternalInput")
    winit = nc.dram_tensor("winit", [K, B_LOC], bf16, kind="ExternalInput")
    colsA = nc.dram_tensor("colsA", [K, 4], f32, kind="ExternalInput")
    # colsA columns: 0=b bias, 1=exp(start), 2=iota, 3=ones(f32)
    onesb = nc.dram_tensor("onesb", [K, 2], bf16, kind="ExternalInput")
    # onesb columns: 0=s0 (colsum stationary), 1=ones (dot stationary)
    onesrow = nc.dram_tensor("onesrow", [1, K], bf16, kind="ExternalInput")
    out_d = nc.dram_tensor("out", [1, B_LOC], f32, kind="ExternalOutput")

    AF = mybir.ActivationFunctionType
    ALU = mybir.AluOpType
    AX = mybir.AxisListType

    with tile.TileContext(nc) as tc:
        with (
            tc.tile_pool(name="consts", bufs=1) as consts,
            tc.tile_pool(name="persist", bufs=1) as persist,
            tc.tile_pool(name="small", bufs=4) as small,
            tc.tile_pool(name="alpha", bufs=4) as apool,
            tc.tile_pool(name="xb", bufs=4) as xpool,
        ):
            # ---- constants ----
            w_sb = consts.tile([128, DC, K], bf16)
            nc.scalar.dma_start(w_sb[:], wq[:])
            mf_sb = consts.tile([K, K], bf16)
            nc.scalar.dma_start(mf_sb[:], mf[:])
            mb_sb = consts.tile([K, K], bf16)
            nc.scalar.dma_start(mb_sb[:], mb[:])
            tag_sb = consts.tile([1, BT], bf16)
            nc.scalar.dma_start(tag_sb[:], tagr[:])
            winit_sb = consts.tile([K, B_LOC], bf16)
            nc.scalar.dma_start(winit_sb[:], winit[:])
            colsA_sb = consts.tile([K, 4], f32)
            nc.scalar.dma_start(colsA_sb[:], colsA[:])
            onesb_sb = consts.tile([K, 2], bf16)
            nc.scalar.dma_start(onesb_sb[:], onesb[:])
            onesrow_sb = consts.tile([1, K], bf16)
            nc.scalar.dma_start(onesrow_sb[:], onesrow[:])

            bcol = colsA_sb[:, 0:1]
            expstart = colsA_sb[:, 1:2]
            iota = colsA_sb[:, 2:3]
            onesf = colsA_sb[:, 3:4]
            s0col = onesb_sb[:, 0:1]
            onescol = onesb_sb[:, 1:2]

            # ---- persistent tensors ----
            hid_sb = persist.tile([128, B_LOC, DC, T], bf16)
            E2 = persist.tile([K, B_LOC, T], bf16)  # scaled E'
            Eraw = persist.tile([K, B_LOC, T], bf16)
            emis = persist.tile([K, B_LOC, T], bf16)
            lncs = persist.tile([1, B_LOC, T], f32)
            lnsums = persist.tile([1, B_LOC], f32)
            goldk = persist.tile([K, B_LOC], f32)
            scr = persist.tile([K, T], bf16)  # scatter target for stt

            # ---- prep: DMA, GEMM, E', gold ----
            # Phase-ordered so the ScalarE activation table loads only 4x
            # (Exp / Ln / Exp / final Ln) instead of thrashing per sequence.
            with (
                tc.tile_pool(name="pe_ps", bufs=3, space=bass.MemorySpace.PSUM) as pe_ps,
                tc.tile_pool(name="cs_ps", bufs=2, space=bass.MemorySpace.PSUM) as cs_ps,
                tc.tile_pool(name="bc_ps", bufs=2, space=bass.MemorySpace.PSUM) as bc_ps,
                tc.tile_pool(name="g_ps", bufs=1, space=bass.MemorySpace.PSUM) as g_ps,
            ):
              # chunked DMAs in sequence order (GpSimd dispatch is ~25ns/DMA)
              # so GEMM(s) can start as soon as sequence s lands.
              for s in range(B_LOC):
                for dc in range(DC):
                    nc.gpsimd.dma_start(hid_sb[:, s, dc, :], hidT[:, s, dc, :])
              for s in range(B_LOC):
                ps_e = pe_ps.tile([K, T], f32, tag="pse")
                for dc in range(DC):
                    nc.tensor.matmul(
                        ps_e[:],
                        w_sb[:, dc, :],
                        hid_sb[:, s, dc, :],
                        start=(dc == 0),
                        stop=(dc == DC - 1),
                    )
                # E_raw = exp(emis + b); raw emissions kept for the gold score
                nc.scalar.activation(Eraw[:, s, :], ps_e[:], AF.Exp, bias=bcol)
                nc.vector.tensor_scalar_add(emis[:, s, :], ps_e[:], bcol)
                # cs = s0 * colsum(E_raw)
                ps_cs = cs_ps.tile([1, T], f32, tag="cs", name=f"cs{s}")
                nc.tensor.matmul(ps_cs[:], s0col, Eraw[:, s, :], start=True, stop=True)
                # ln(cs) with fused free-dim sum; ring of 2 PSUM bufs makes
                # colsum(s+2) wait on LN(s) - fine, they ping-pong.
                nc.scalar.activation(
                    lncs[:, s, :], ps_cs[:], AF.Ln,
                    accum_out=lnsums[:, s : s + 1],
                )
              # scale rows 1/cs = exp(-ln cs) on ScalarE (DVE reciprocal is
              # ~3.3us per row); one table switch back to Exp for all 8.
              for s in range(B_LOC):
                rcs = small.tile([1, T], bf16, tag="rcs", name=f"rcs{s}")
                nc.scalar.activation(rcs[:], lncs[:, s, :], AF.Exp, scale=-1.0)
                ps_bc = bc_ps.tile([K, T], f32, tag="bc")
                nc.tensor.matmul(ps_bc[:], onesrow_sb[:], rcs[:], start=True, stop=True)
                nc.vector.tensor_mul(E2[:, s, :], Eraw[:, s, :], ps_bc[:])
                # gold emissions: bcast tags, compare to iota, dot with emis
                ps_t = bc_ps.tile([K, T], f32, tag="bc")
                nc.tensor.matmul(
                    ps_t[:], onesrow_sb[:], tag_sb[:, s * T : (s + 1) * T],
                    start=True, stop=True,
                )
                nc.vector.scalar_tensor_tensor(
                    scr[:],
                    ps_t[:],
                    iota,
                    emis[:, s, :],
                    ALU.is_equal,
                    ALU.mult,
                    accum_out=goldk[:, s : s + 1],
                )
              # gold tag-sum reduction, off the scan critical path
              ps_g = g_ps.tile([1, B_LOC], f32, tag="g")
              nc.tensor.matmul(ps_g[:], onesf, goldk[:], start=True, stop=True)
              goldrow = small.tile([1, B_LOC], f32, tag="grow")
              nc.vector.tensor_copy(goldrow[:], ps_g[:])

            # ---- forward/backward scan ----
            with (
                tc.tile_pool(name="sf_ps", bufs=3, space=bass.MemorySpace.PSUM) as sf_ps,
                tc.tile_pool(name="sb_ps", bufs=3, space=bass.MemorySpace.PSUM) as sb_ps,
                tc.tile_pool(name="z_ps", bufs=2, space=bass.MemorySpace.PSUM) as z_ps,
            ):
              alpha = apool.tile([K, B_LOC], bf16, tag="a")
              nc.vector.tensor_scalar_mul(alpha[:], E2[:, :, 0], expstart)
              alpha_ap = alpha[:]
              w_ap = winit_sb[:]

              for i in range(1, MID):
                tf = i
                tb = T - i
                ps_f = sf_ps.tile([K, B_LOC], f32, tag="psf", name=f"pf{i}")
                nc.tensor.matmul(ps_f[:], mf_sb[:], alpha_ap, start=True, stop=True)
                x_b = xpool.tile([K, B_LOC], bf16, tag="x", name=f"xb{i}")
                nc.vector.tensor_mul(x_b[:], w_ap, E2[:, :, tb])
                ps_b = sb_ps.tile([K, B_LOC], f32, tag="psb", name=f"pb{i}")
                nc.tensor.matmul(ps_b[:], mb_sb[:], x_b[:], start=True, stop=True)
                alpha_new = apool.tile([K, B_LOC], bf16, tag="a", name=f"al{i}")
                nc.vector.tensor_mul(alpha_new[:], ps_f[:], E2[:, :, tf])
                alpha_ap = alpha_new[:]
                w_ap = ps_b[:]

              # tail: bwd needs one more step (t = MID)
              x_l = xpool.tile([K, B_LOC], bf16, tag="x", name="xlast")
              nc.vector.tensor_mul(x_l[:], w_ap, E2[:, :, MID])
              ps_l = sb_ps.tile([K, B_LOC], f32, tag="psb", name="pblast")
              nc.tensor.matmul(ps_l[:], mb_sb[:], x_l[:], start=True, stop=True)

              # ---- epilogue: log_Z = ln(w . a) + sum ln(cs); out = log_Z - goldE
              wdot = small.tile([K, B_LOC], bf16, tag="wdot")
              nc.vector.tensor_mul(wdot[:], ps_l[:], alpha_ap)
              ps_z = z_ps.tile([1, B_LOC], f32, tag="z")
              nc.tensor.matmul(ps_z[:], onescol, wdot[:], start=True, stop=True)
              lnz = small.tile([1, B_LOC], f32, tag="row")
              nc.scalar.activation(lnz[:], ps_z[:], AF.Ln)
              acc = small.tile([1, B_LOC], f32, tag="row")
              nc.vector.tensor_add(acc[:], lnz[:], lnsums[:])
              outrow = small.tile([1, B_LOC], f32, tag="row")
              nc.vector.tensor_sub(outrow[:], acc[:], goldrow[:])
              nc.sync.dma_start(out_d[:], outrow[:])

    nc.compile()
    return nc


def _get_compiled():
    if "nc" not in _COMPILED:
        _COMPILED["nc"] = _build()
    return _COMPILED["nc"]


def _host_inputs(full_hidden, tag_ids, W, b, transitions, start_trans, end_trans):
    """Build the per-core in_maps plus host-side gold transition scores."""
    import ml_dtypes

    bf16 = ml_dtypes.bfloat16

    full_hidden = np.asarray(full_hidden, dtype=np.float32)
    tags = np.asarray(tag_ids).astype(np.int64)
    W = np.asarray(W, dtype=np.float32)
    b = np.asarray(b, dtype=np.float32)
    transitions = np.asarray(transitions, dtype=np.float32)
    start_trans = np.asarray(start_trans, dtype=np.float32)
    end_trans = np.asarray(end_trans, dtype=np.float32)

    M = np.exp(transitions)
    s0 = float(M.mean())

    common = {
        "wq": np.ascontiguousarray(
            W.reshape(DC, 128, K).transpose(1, 0, 2)
        ).astype(bf16),
        "mf": M.astype(bf16),
        "mb": np.ascontiguousarray(M.T).astype(bf16),
        "winit": np.tile(
            np.exp(end_trans)[:, None].astype(np.float32), (1, B_LOC)
        ).astype(bf16),
        "colsA": np.ascontiguousarray(
            np.stack(
                [b, np.exp(start_trans), np.arange(K, dtype=np.float32),
                 np.ones(K, np.float32)],
                axis=1,
            )
        ),
        "onesb": np.ascontiguousarray(
            np.stack(
                [np.full(K, s0, np.float32), np.ones(K, np.float32)], axis=1
            )
        ).astype(bf16),
        "onesrow": np.ones((1, K), np.float32).astype(bf16),
    }

    in_maps = []
    for c in range(N_CORES):
        sl = slice(c * B_LOC, (c + 1) * B_LOC)
        h = full_hidden[sl]  # [8, 512, 1024]
        hidT = np.ascontiguousarray(
            h.reshape(B_LOC, T, DC, 128).transpose(3, 0, 2, 1)
        ).astype(bf16)  # [128, seq, dc, t]
        in_maps.append(
            {
                "hidT": hidT,
                "tagr": tags[sl].astype(np.float32).reshape(1, BT).astype(bf16),
                **common,
            }
        )

    # host part of the gold score: transitions + start/end (tags only)
    gold_trans = (
        transitions[tags[:, :-1], tags[:, 1:]].sum(axis=1)
        + start_trans[tags[:, 0]]
        + end_trans[tags[:, -1]]
    ).astype(np.float32)
    return in_maps, gold_trans


def kernel(full_hidden, tag_ids, mask, W, b, transitions, start_trans, end_trans):
    global LAST_RESULT
    from concourse.bass_utils import run_bass_kernel_spmd

    in_maps, gold_trans = _host_inputs(
        full_hidden, tag_ids, W, b, transitions, start_trans, end_trans
    )
    nc = _get_compiled()
    res = run_bass_kernel_spmd(nc, in_maps, core_ids=list(range(N_CORES)))
    LAST_RESULT = res
    dev = np.concatenate(
        [np.asarray(res.results[c]["out"]).reshape(B_LOC) for c in range(N_CORES)]
    ).astype(np.float32)
    return dev - gold_trans
